# revision 2
# baseline (speedup 1.0000x reference)
"""Trainium2 Bass kernel for additive (Bahdanau) attention.

reference computation (B=4, Q=K=512, D=256, H=128, V=256):
    fq = queries @ wq_w.T + wq_b                    # [B,Q,H]
    fk = keys @ wk_w.T + wk_b                       # [B,K,H]
    scores[b,q,k] = sum_h wv[h]*tanh(fq[b,q,h]+fk[b,k,h]) + wv_b
    attn = softmax(mask(scores, valid_lens), axis=k)
    out  = attn @ values                            # [B,Q,V]

Sharding: 8 cores = 4 batches x 2 query-halves. Each core owns 256 query
rows of one batch and needs the full keys/values of that batch -> zero
cross-core communication.

Per-core device algorithm (H=128 on partitions):
  - fqT[h,q], fkT[h,k] via PE matmuls (inputs pre-transposed host-side).
  - per q: DVE tensor_scalar add (per-partition scalar fqT[:,q]) builds
    pre[h,k] = fkT + fq; batched 16 q's per ACT tanh -> T bf16.
  - scores block [128q, 512k] accumulated in PSUM: first a rank-1 matmul
    writes the additive mask row (start=True), then per q one matmul with
    a one-hot-weighted wv column block (lhsT = wv (x) e_j) adds row q.
  - softmax without max-subtraction (scores bounded by sum|wv| ~ 9):
    ACT exp with accum_out giving row sums; masked lanes underflow to 0.
  - attn^T via PE transposes, attn @ values via PE, scaled by 1/denom.
"""

import sys

sys.path.insert(0, "/opt/trn_rl_repo")

from contextlib import ExitStack

import numpy as np

from concourse import bacc, mybir, tile
from concourse.bass_utils import run_bass_kernel_spmd
from concourse.masks import make_identity

B, Q, K, D, H, V = 4, 512, 512, 256, 128, 256
QS = Q // 2          # query rows per core
NCORES = 8
MASK_VALUE = -1000000.0
ST = 16              # q rows per tanh supertile
NBLK = QS // 128     # 128-q score blocks per core
NST = 128 // ST      # supertiles per block

f32 = mybir.dt.float32
bf16 = mybir.dt.bfloat16


def _build_graph(nc, tc, ctx, tensors):
    qT_d, kT_d, val_d, wqT_d, wkT_d, wb_d, z_d, m_d, out_d = tensors
    Tanh = mybir.ActivationFunctionType.Tanh
    Exp = mybir.ActivationFunctionType.Exp

    cpool = ctx.enter_context(tc.tile_pool(name="const", bufs=1))
    inp = ctx.enter_context(tc.tile_pool(name="inp", bufs=1))
    prep = ctx.enter_context(tc.tile_pool(name="prep", bufs=2))
    ttp = ctx.enter_context(tc.tile_pool(name="ttp", bufs=2))
    smp = ctx.enter_context(tc.tile_pool(name="smp", bufs=2))
    outp = ctx.enter_context(tc.tile_pool(name="outp", bufs=2))
    ps_big = ctx.enter_context(tc.tile_pool(name="ps_big", bufs=2, space="PSUM"))
    ps_tr = ctx.enter_context(tc.tile_pool(name="ps_tr", bufs=2, space="PSUM"))
    ps_av = ctx.enter_context(tc.tile_pool(name="ps_av", bufs=2, space="PSUM"))

    # ---------------- loads ----------------
    kT_sb = []
    for i in range(2):
        t = inp.tile([128, K], f32, tag=f"kT{i}", name=f"kT_sb{i}")
        nc.sync.dma_start(t[:], kT_d[i * 128:(i + 1) * 128, :])
        kT_sb.append(t)
    qT_sb = []
    for i in range(2):
        t = inp.tile([128, QS], f32, tag=f"qT{i}", name=f"qT_sb{i}")
        nc.sync.dma_start(t[:], qT_d[i * 128:(i + 1) * 128, :])
        qT_sb.append(t)
    wqT_sb, wkT_sb = [], []
    for i in range(2):
        t = inp.tile([128, H], f32, tag=f"wqT{i}", name=f"wqT_sb{i}")
        nc.sync.dma_start(t[:], wqT_d[i * 128:(i + 1) * 128, :])
        wqT_sb.append(t)
        t = inp.tile([128, H], f32, tag=f"wkT{i}", name=f"wkT_sb{i}")
        nc.sync.dma_start(t[:], wkT_d[i * 128:(i + 1) * 128, :])
        wkT_sb.append(t)
    wqkb = cpool.tile([H, 1], f32, tag="wqkb")
    nc.sync.dma_start(wqkb[:], wb_d[:])

    z32f = inp.tile([128, 1024], f32, tag="z32f")
    nc.sync.dma_start(z32f[:], z_d[:])
    z32 = cpool.tile([128, 1024], bf16, tag="z32")
    nc.vector.tensor_copy(z32[:], z32f[:])

    mask_f = inp.tile([1, K], f32, tag="maskf")
    nc.sync.dma_start(mask_f[:], m_d[:])
    mask_bf = cpool.tile([1, K], bf16, tag="maskbf")
    nc.vector.tensor_copy(mask_bf[:], mask_f[:])

    ones_bf = cpool.tile([1, H], bf16, tag="ones")
    nc.vector.memset(ones_bf[:], 1.0)

    ident = cpool.tile([128, 128], bf16, tag="ident")
    make_identity(nc, ident[:])

    vbf = cpool.tile([128, 4 * V], bf16, tag="vbf")
    for ci in range(4):
        t = inp.tile([128, V], f32, tag=f"valf{ci % 2}", name=f"val{ci}")
        nc.sync.dma_start(t[:], val_d[ci * 128:(ci + 1) * 128, :])
        nc.vector.tensor_copy(vbf[:, ci * V:(ci + 1) * V], t[:])

    # ---------------- projections ----------------
    fk_ps = ps_big.tile([128, K], f32, tag="big", name="fk_ps")
    nc.tensor.matmul(fk_ps[:], wkT_sb[0][:], kT_sb[0][:], start=True, stop=False)
    nc.tensor.matmul(fk_ps[:], wkT_sb[1][:], kT_sb[1][:], start=False, stop=True)
    fk_sb = cpool.tile([128, K], f32, tag="fk_sb")
    nc.vector.tensor_copy(fk_sb[:], fk_ps[:])

    fq_ps = ps_big.tile([128, QS], f32, tag="big", name="fq_ps")
    nc.tensor.matmul(fq_ps[:], wqT_sb[0][:], qT_sb[0][:], start=True, stop=False)
    nc.tensor.matmul(fq_ps[:], wqT_sb[1][:], qT_sb[1][:], start=False, stop=True)
    fq_sb = cpool.tile([128, QS], f32, tag="fq_sb")
    # fold wq_b + wk_b into fq so the tanh input needs no extra bias
    nc.vector.tensor_scalar_add(fq_sb[:], fq_ps[:], wqkb[:])

    # ---------------- main loop ----------------
    for blk in range(NBLK):
        sc_ps = ps_big.tile([128, K], f32, tag="big", name=f"sc{blk}")
        # rank-1 matmul writes mask row to every q-partition, clears bank
        nc.tensor.matmul(sc_ps[:], ones_bf[:], mask_bf[:], start=True,
                         stop=False, skip_group_check=True)
        for st in range(NST):
            pre = prep.tile([128, ST * K], f32, tag="pre", name=f"pre{blk}_{st}")
            for i in range(ST):
                q = blk * 128 + st * ST + i
                nc.vector.tensor_scalar_add(
                    pre[:, i * K:(i + 1) * K], fk_sb[:], fq_sb[:, q:q + 1])
            tt = ttp.tile([128, ST * K], bf16, tag="tt", name=f"tt{blk}_{st}")
            nc.scalar.activation(tt[:], pre[:], Tanh)
            for i in range(ST):
                r = st * ST + i
                g, j = divmod(r, 32)
                nc.tensor.matmul(
                    sc_ps[g * 32:(g + 1) * 32, :],
                    z32[:, j * 32:(j + 1) * 32],
                    tt[:, i * K:(i + 1) * K],
                    start=False, stop=(r == 127), skip_group_check=True,
                    tile_position=(0, g * 32))

        E = smp.tile([128, K], bf16, tag="E", name=f"E{blk}")
        denom = smp.tile([128, 1], f32, tag="denom", name=f"denom{blk}")
        nc.scalar.activation(E[:], sc_ps[:], Exp, accum_out=denom[:])
        recip = smp.tile([128, 1], f32, tag="recip", name=f"recip{blk}")
        nc.vector.reciprocal(recip[:], denom[:])

        ET = smp.tile([128, K], bf16, tag="ET", name=f"ET{blk}")
        for ci in range(4):
            tp = ps_tr.tile([128, 128], bf16, tag="tr", name=f"tr{blk}_{ci}")
            nc.tensor.transpose(tp[:], E[:, ci * 128:(ci + 1) * 128], ident[:])
            nc.vector.tensor_copy(ET[:, ci * 128:(ci + 1) * 128], tp[:])

        av = ps_av.tile([128, V], f32, tag="av", name=f"av{blk}")
        for ci in range(4):
            nc.tensor.matmul(av[:], ET[:, ci * 128:(ci + 1) * 128],
                             vbf[:, ci * V:(ci + 1) * V],
                             start=(ci == 0), stop=(ci == 3))
        osb = outp.tile([128, V], f32, tag="osb", name=f"osb{blk}")
        nc.vector.tensor_scalar_mul(osb[:], av[:], recip[:])
        nc.sync.dma_start(out_d[blk * 128:(blk + 1) * 128, :], osb[:])


def _build_kernel():
    nc = bacc.Bacc("TRN2", target_bir_lowering=False, debug=False,
                   num_devices=NCORES)
    qT_d = nc.dram_tensor("qT", [D, QS], f32, kind="ExternalInput")
    kT_d = nc.dram_tensor("kT", [D, K], f32, kind="ExternalInput")
    val_d = nc.dram_tensor("vals", [K, V], f32, kind="ExternalInput")
    wqT_d = nc.dram_tensor("wqT", [D, H], f32, kind="ExternalInput")
    wkT_d = nc.dram_tensor("wkT", [D, H], f32, kind="ExternalInput")
    wb_d = nc.dram_tensor("wqkb", [H, 1], f32, kind="ExternalInput")
    z_d = nc.dram_tensor("z32f", [H, 1024], f32, kind="ExternalInput")
    m_d = nc.dram_tensor("maskrow", [1, K], f32, kind="ExternalInput")
    out_d = nc.dram_tensor("out", [QS, V], f32, kind="ExternalOutput")
    tensors = (qT_d, kT_d, val_d, wqT_d, wkT_d, wb_d, z_d, m_d, out_d)

    with tile.TileContext(nc) as tc, ExitStack() as ctx:
        _build_graph(nc, tc, ctx, tensors)
    nc.compile()
    return nc


_NC = None


def _get_nc():
    global _NC
    if _NC is None:
        _NC = _build_kernel()
    return _NC


def prepare_in_maps(queries, keys, values, valid_lens, wq_w, wq_b, wk_w,
                    wk_b, wv_w, wv_b):
    queries = np.asarray(queries, np.float32)
    keys = np.asarray(keys, np.float32)
    values = np.asarray(values, np.float32)
    wq_w = np.asarray(wq_w, np.float32)
    wq_b = np.asarray(wq_b, np.float32)
    wk_w = np.asarray(wk_w, np.float32)
    wk_b = np.asarray(wk_b, np.float32)
    wv_w = np.asarray(wv_w, np.float32)
    wv_b = np.asarray(wv_b, np.float32)
    valid_lens = np.asarray(valid_lens)

    wqT = np.ascontiguousarray(wq_w.T)
    wkT = np.ascontiguousarray(wk_w.T)
    wqkb = (wq_b + wk_b).reshape(H, 1).astype(np.float32)
    wv = wv_w.reshape(H)
    # one-hot blocks: z[h, j*32+m] = wv[h] iff m == j
    z = np.zeros((H, 1024), np.float32)
    for j in range(32):
        z[:, j * 33] = wv

    in_maps = []
    for c in range(NCORES):
        b, half = divmod(c, 2)
        vl = int(valid_lens[b])
        mask = np.full((1, K), MASK_VALUE, np.float32)
        mask[0, :vl] = 0.0
        mask += np.float32(wv_b.reshape(-1)[0])
        in_maps.append({
            "qT": np.ascontiguousarray(queries[b, half * QS:(half + 1) * QS, :].T),
            "kT": np.ascontiguousarray(keys[b].T),
            "vals": np.ascontiguousarray(values[b]),
            "wqT": wqT, "wkT": wkT, "wqkb": wqkb,
            "z32f": z, "maskrow": mask,
        })
    return in_maps


def assemble_out(results):
    out = np.empty((B, Q, V), np.float32)
    for c in range(NCORES):
        b, half = divmod(c, 2)
        out[b, half * QS:(half + 1) * QS, :] = results[c]["out"]
    return out


def kernel(**inputs):
    nc = _get_nc()
    in_maps = prepare_in_maps(**inputs)
    res = run_bass_kernel_spmd(nc, in_maps, list(range(NCORES))).results
    return assemble_out(res)


if __name__ == "__main__":
    rng = np.random.default_rng(0)
    inp = {
        "queries": rng.standard_normal((B, Q, D), np.float32),
        "keys": rng.standard_normal((B, K, D), np.float32),
        "values": rng.standard_normal((B, K, V), np.float32),
        "valid_lens": rng.integers(1, K + 1, (B,)).astype(np.int32),
        "wq_w": (rng.standard_normal((H, D), np.float32) / np.sqrt(D)).astype(np.float32),
        "wq_b": np.zeros((H,), np.float32),
        "wk_w": (rng.standard_normal((H, D), np.float32) / np.sqrt(D)).astype(np.float32),
        "wk_b": np.zeros((H,), np.float32),
        "wv_w": (rng.standard_normal((1, H), np.float32) / np.sqrt(H)).astype(np.float32),
        "wv_b": np.zeros((1,), np.float32),
    }
    out = kernel(**inp)
    print("kernel output", out.shape, out.dtype, float(np.abs(out).mean()))


# revision 3
# speedup vs baseline: 1.3376x; 1.3376x over previous
"""Trainium2 Bass kernel for additive (Bahdanau) attention.

reference computation (B=4, Q=K=512, D=256, H=128, V=256):
    fq = queries @ wq_w.T + wq_b                    # [B,Q,H]
    fk = keys @ wk_w.T + wk_b                       # [B,K,H]
    scores[b,q,k] = sum_h wv[h]*tanh(fq[b,q,h]+fk[b,k,h]) + wv_b
    attn = softmax(mask(scores, valid_lens), axis=k)
    out  = attn @ values                            # [B,Q,V]

Sharding: 8 cores = 4 batches x 2 query-halves; zero cross-core traffic.

Key specialization: keys at positions >= valid_len get attention weight
exactly 0 (additive -1e6 mask -> exp underflows to 0.0f), so the kernel
is compiled for KC = ceil(max(valid_lens)/32)*32 key positions only.
This is exact, input-adaptive (compile cache per KC), and cuts the
dominant per-element tanh work proportionally.

Per-core device algorithm (H=128 on partitions):
  - fqT[h,q], fkT[h,k] via PE matmuls (inputs pre-transposed host-side,
    all small inputs packed into two DMA transfers).
  - per q: DVE tensor_scalar add (per-partition scalar fqT[:,q]) builds
    pre[h,k] = fkT + fq; batched up to 16 q per ACT tanh -> T bf16.
    Supertile sizes ramp 4,4,8,...16...,8,4,4 to shorten pipeline
    fill/drain on the ACT critical path.
  - scores block [128q, KC] accumulated in PSUM: first a rank-1 matmul
    writes the additive mask row (start=True clears the bank), then per
    q one matmul with a one-hot-weighted wv column (lhsT = wv (x) e_j,
    M=32 col-group tiles) accumulates row q.
  - softmax without max-subtraction (|scores| <= sum|wv| ~ 9, so exp is
    safe in f32): ACT exp with accum_out produces row sums in the same
    instruction; masked lanes underflow to exactly 0.
  - attn^T via PE transposes; attn @ values on PE in bf16; final
    per-row 1/denom scale fused into the PSUM->SBUF copy.
"""

import sys

sys.path.insert(0, "/opt/trn_rl_repo")

from contextlib import ExitStack

import numpy as np

from concourse import bacc, mybir, tile
from concourse.bass_utils import run_bass_kernel_spmd
from concourse.masks import make_identity

B, Q, K, D, H, V = 4, 512, 512, 256, 128, 256
QS = Q // 2          # query rows per core
NCORES = 8
MASK_VALUE = -1000000.0

f32 = mybir.dt.float32
bf16 = mybir.dt.bfloat16

# supertile q-counts per 128-q block: ramp up in block 0, down in last
STS_FIRST = [4, 4, 8] + [16] * 7
STS_LAST = [16] * 7 + [8, 4, 4]


def _build_graph(nc, tc, ctx, tensors, KC):
    p1_d, p2_d, m_d, out_d = tensors
    NKC = (KC + 127) // 128          # 128-row key chunks (last may be partial)
    WLAST = KC - (NKC - 1) * 128     # rows in last chunk
    F1 = 2 * KC + 2 * 256 + 4 * 128 + 1
    Tanh = mybir.ActivationFunctionType.Tanh
    Exp = mybir.ActivationFunctionType.Exp

    cpool = ctx.enter_context(tc.tile_pool(name="const", bufs=1))
    inp = ctx.enter_context(tc.tile_pool(name="inp", bufs=1))
    prep = ctx.enter_context(tc.tile_pool(name="prep", bufs=3))
    ttp = ctx.enter_context(tc.tile_pool(name="ttp", bufs=3))
    smp = ctx.enter_context(tc.tile_pool(name="smp", bufs=2))
    outp = ctx.enter_context(tc.tile_pool(name="outp", bufs=2))
    ps_big = ctx.enter_context(tc.tile_pool(name="ps_big", bufs=2, space="PSUM"))
    ps_tr = ctx.enter_context(tc.tile_pool(name="ps_tr", bufs=2, space="PSUM"))
    ps_av = ctx.enter_context(tc.tile_pool(name="ps_av", bufs=2, space="PSUM"))

    # ---------------- loads ----------------
    pk1 = inp.tile([128, F1], f32, tag="pk1")
    nc.sync.dma_start(pk1[:], p1_d[:])
    kT_sb = [pk1[:, i * KC:(i + 1) * KC] for i in range(2)]
    o = 2 * KC
    qT_sb = [pk1[:, o + i * 256:o + (i + 1) * 256] for i in range(2)]
    o += 512
    wqT_sb = [pk1[:, o + i * 128:o + (i + 1) * 128] for i in range(2)]
    o += 256
    wkT_sb = [pk1[:, o + i * 128:o + (i + 1) * 128] for i in range(2)]
    o += 256
    wqkb = pk1[:, o:o + 1]

    mask_f = inp.tile([1, KC], f32, tag="maskf")
    nc.sync.dma_start(mask_f[:], m_d[:])
    mask_bf = cpool.tile([1, KC], bf16, tag="maskbf")
    nc.vector.tensor_copy(mask_bf[:], mask_f[:])

    F2 = 1024 + NKC * 256
    pk2 = inp.tile([128, F2], f32, tag="pk2")
    nc.sync.dma_start(pk2[:], p2_d[:])
    z32f = pk2[:, 0:1024]
    z32 = cpool.tile([128, 1024], bf16, tag="z32")
    nc.vector.tensor_copy(z32[:], z32f[:])
    vbf = cpool.tile([128, NKC * V], bf16, tag="vbf")
    nc.vector.tensor_copy(vbf[:], pk2[:, 1024:1024 + NKC * V])

    ones_bf = cpool.tile([1, H], bf16, tag="ones")
    nc.vector.memset(ones_bf[:], 1.0)

    ident = cpool.tile([128, 128], bf16, tag="ident")
    make_identity(nc, ident[:])

    # ---------------- projections ----------------
    fk_ps = ps_big.tile([128, K], f32, tag="big", name="fk_ps")
    nc.tensor.matmul(fk_ps[:, :KC], wkT_sb[0], kT_sb[0], start=True, stop=False)
    nc.tensor.matmul(fk_ps[:, :KC], wkT_sb[1], kT_sb[1], start=False, stop=True)
    fk_sb = cpool.tile([128, KC], f32, tag="fk_sb")
    nc.vector.tensor_copy(fk_sb[:], fk_ps[:, :KC])

    fq_ps = ps_big.tile([128, QS], f32, tag="big", name="fq_ps")
    nc.tensor.matmul(fq_ps[:], wqT_sb[0], qT_sb[0], start=True, stop=False)
    nc.tensor.matmul(fq_ps[:], wqT_sb[1], qT_sb[1], start=False, stop=True)
    fq_sb = cpool.tile([128, QS], f32, tag="fq_sb")
    # fold wq_b + wk_b into fq so the tanh input needs no extra bias
    nc.vector.tensor_scalar_add(fq_sb[:], fq_ps[:], wqkb)

    # ---------------- main loop ----------------
    for blk in range(2):
        sts = STS_FIRST if blk == 0 else STS_LAST
        sc_ps = ps_big.tile([128, K], f32, tag="big", name=f"sc{blk}")
        # rank-1 matmul writes mask row to every q-partition, clears bank
        nc.tensor.matmul(sc_ps[:, :KC], ones_bf[:], mask_bf[:], start=True,
                         stop=False, skip_group_check=True)
        r = 0
        for st, stq in enumerate(sts):
            pre = prep.tile([128, 16 * KC], f32, tag="pre", name=f"pre{blk}_{st}")
            for i in range(stq):
                q = blk * 128 + r + i
                nc.vector.tensor_scalar_add(
                    pre[:, i * KC:(i + 1) * KC], fk_sb[:], fq_sb[:, q:q + 1])
            tt = ttp.tile([128, 16 * KC], bf16, tag="tt", name=f"tt{blk}_{st}")
            nc.scalar.activation(tt[:, :stq * KC], pre[:, :stq * KC], Tanh)
            for i in range(stq):
                g, j = divmod(r + i, 32)
                nc.tensor.matmul(
                    sc_ps[g * 32:(g + 1) * 32, :KC],
                    z32[:, j * 32:(j + 1) * 32],
                    tt[:, i * KC:(i + 1) * KC],
                    start=False, stop=(r + i == 127), skip_group_check=True,
                    tile_position=(0, g * 32))
            r += stq

        E = smp.tile([128, KC], bf16, tag="E", name=f"E{blk}")
        denom = smp.tile([128, 1], f32, tag="denom", name=f"denom{blk}")
        nc.scalar.activation(E[:], sc_ps[:, :KC], Exp, accum_out=denom[:])
        recip = smp.tile([128, 1], f32, tag="recip", name=f"recip{blk}")
        nc.vector.reciprocal(recip[:], denom[:])

        ET = smp.tile([128, NKC * 128], bf16, tag="ET", name=f"ET{blk}")
        for ci in range(NKC):
            w = 128 if ci < NKC - 1 else WLAST
            tp = ps_tr.tile([128, 128], bf16, tag="tr", name=f"tr{blk}_{ci}")
            nc.tensor.transpose(tp[:w, :], E[:, ci * 128:ci * 128 + w], ident[:])
            nc.vector.tensor_copy(ET[:w, ci * 128:(ci + 1) * 128], tp[:w, :])

        av = ps_av.tile([128, V], f32, tag="av", name=f"av{blk}")
        for ci in range(NKC):
            w = 128 if ci < NKC - 1 else WLAST
            nc.tensor.matmul(av[:], ET[:w, ci * 128:(ci + 1) * 128],
                             vbf[:w, ci * V:(ci + 1) * V],
                             start=(ci == 0), stop=(ci == NKC - 1))
        osb = outp.tile([128, V], f32, tag="osb", name=f"osb{blk}")
        nc.vector.tensor_scalar_mul(osb[:], av[:], recip[:])
        nc.sync.dma_start(out_d[blk * 128:(blk + 1) * 128, :], osb[:])


def _build_kernel(KC):
    NKC = (KC + 127) // 128
    F1 = 2 * KC + 2 * 256 + 4 * 128 + 1
    F2 = 1024 + NKC * 256
    nc = bacc.Bacc("TRN2", target_bir_lowering=False, debug=False,
                   num_devices=NCORES)
    p1_d = nc.dram_tensor("pack1", [128, F1], f32, kind="ExternalInput")
    p2_d = nc.dram_tensor("pack2", [128, F2], f32, kind="ExternalInput")
    m_d = nc.dram_tensor("maskrow", [1, KC], f32, kind="ExternalInput")
    out_d = nc.dram_tensor("out", [QS, V], f32, kind="ExternalOutput")

    with tile.TileContext(nc) as tc, ExitStack() as ctx:
        _build_graph(nc, tc, ctx, (p1_d, p2_d, m_d, out_d), KC)
    nc.compile()
    return nc


_NC_CACHE = {}


def _get_nc(KC):
    if KC not in _NC_CACHE:
        _NC_CACHE[KC] = _build_kernel(KC)
    return _NC_CACHE[KC]


def _choose_kc(valid_lens):
    mx = int(np.max(valid_lens))
    mx = max(32, min(K, mx))
    return (mx + 31) // 32 * 32


def prepare_in_maps(queries, keys, values, valid_lens, wq_w, wq_b, wk_w,
                    wk_b, wv_w, wv_b):
    queries = np.asarray(queries, np.float32)
    keys = np.asarray(keys, np.float32)
    values = np.asarray(values, np.float32)
    wq_w = np.asarray(wq_w, np.float32)
    wq_b = np.asarray(wq_b, np.float32)
    wk_w = np.asarray(wk_w, np.float32)
    wk_b = np.asarray(wk_b, np.float32)
    wv_w = np.asarray(wv_w, np.float32)
    wv_b = np.asarray(wv_b, np.float32)
    valid_lens = np.asarray(valid_lens)

    KC = _choose_kc(valid_lens)
    NKC = (KC + 127) // 128

    wqT = wq_w.T                     # [D, H]
    wkT = wk_w.T
    wqkb = np.repeat((wq_b + wk_b).reshape(H, 1), 1, axis=1)
    wv = wv_w.reshape(H)
    # one-hot blocks: z[h, j*32+m] = wv[h] iff m == j
    z = np.zeros((H, 1024), np.float32)
    for j in range(32):
        z[:, j * 33] = wv

    in_maps = []
    for c in range(NCORES):
        b, half = divmod(c, 2)
        vl = int(valid_lens[b])
        mask = np.full((1, KC), MASK_VALUE, np.float32)
        mask[0, :vl] = 0.0
        mask += np.float32(wv_b.reshape(-1)[0])

        kT = keys[b, :KC, :].T                       # [D, KC]
        qT = queries[b, half * QS:(half + 1) * QS, :].T   # [D, QS]
        pack1 = np.concatenate([
            kT[0:128], kT[128:256],
            qT[0:128], qT[128:256],
            wqT[0:128], wqT[128:256],
            wkT[0:128], wkT[128:256],
            wqkb,
        ], axis=1).astype(np.float32)

        vpad = np.zeros((NKC * 128, V), np.float32)
        vpad[:min(KC, K)] = values[b, :KC, :]
        vchunks = [vpad[ci * 128:(ci + 1) * 128] for ci in range(NKC)]
        pack2 = np.concatenate([z] + vchunks, axis=1).astype(np.float32)

        in_maps.append({
            "pack1": np.ascontiguousarray(pack1),
            "pack2": np.ascontiguousarray(pack2),
            "maskrow": mask,
        })
    return KC, in_maps


def assemble_out(results):
    out = np.empty((B, Q, V), np.float32)
    for c in range(NCORES):
        b, half = divmod(c, 2)
        out[b, half * QS:(half + 1) * QS, :] = results[c]["out"]
    return out


def kernel(**inputs):
    KC, in_maps = prepare_in_maps(**inputs)
    nc = _get_nc(KC)
    res = run_bass_kernel_spmd(nc, in_maps, list(range(NCORES))).results
    return assemble_out(res)


if __name__ == "__main__":
    rng = np.random.default_rng(0)
    inp = {
        "queries": rng.standard_normal((B, Q, D), np.float32),
        "keys": rng.standard_normal((B, K, D), np.float32),
        "values": rng.standard_normal((B, K, V), np.float32),
        "valid_lens": rng.integers(1, K + 1, (B,)).astype(np.int32),
        "wq_w": (rng.standard_normal((H, D), np.float32) / np.sqrt(D)).astype(np.float32),
        "wq_b": np.zeros((H,), np.float32),
        "wk_w": (rng.standard_normal((H, D), np.float32) / np.sqrt(D)).astype(np.float32),
        "wk_b": np.zeros((H,), np.float32),
        "wv_w": (rng.standard_normal((1, H), np.float32) / np.sqrt(H)).astype(np.float32),
        "wv_b": np.zeros((1,), np.float32),
    }
    out = kernel(**inp)
    print("kernel output", out.shape, out.dtype, float(np.abs(out).mean()))


# revision 5
# speedup vs baseline: 1.3922x; 1.0408x over previous
"""Trainium2 Bass kernel for additive (Bahdanau) attention.

reference computation (B=4, Q=K=512, D=256, H=128, V=256):
    fq = queries @ wq_w.T + wq_b                    # [B,Q,H]
    fk = keys @ wk_w.T + wk_b                       # [B,K,H]
    scores[b,q,k] = sum_h wv[h]*tanh(fq[b,q,h]+fk[b,k,h]) + wv_b
    attn = softmax(mask(scores, valid_lens), axis=k)
    out  = attn @ values                            # [B,Q,V]

Sharding: 8 cores = 4 batches x 2 query-halves; zero cross-core traffic.

Key specialization: positions >= valid_len get attention weight exactly
0 (additive -1e6 mask -> f32 exp underflows to 0.0), so the graph is
compiled for KC = ceil(max(valid_lens)/32)*32 key positions (compile
cache per KC). Exact and input-adaptive; cuts the dominant per-element
tanh work proportionally.

Per-core device algorithm (H=128 on partitions):
  - fqT[h,q], fkT[h,k] via PE matmuls (inputs pre-transposed host-side,
    packed into one DMA; values in a second DMA).
  - tanh features, ACT-bound: per q a [128h, KC] tile of
    tanh(fkT + fqT[:,q]).  The first 8 q of block 0 run as ACT
    activations with per-partition bias straight out of the fk PSUM
    (no DVE dependency -> ACT starts ~6us earlier); the rest are
    DVE tensor_scalar adds in bf16 (4x mode) batched 16 q per ACT call.
  - scores block [128q, KC] accumulated in PSUM: first a rank-1 matmul
    writes the additive mask row (start=True clears the bank), then per
    q one matmul with a one-hot-weighted wv column (lhsT = wv (x) e_j,
    M=32 col-group tiles, built on device from wv) accumulates row q.
  - softmax without max-subtraction (|scores| <= sum|wv| ~ 9): ACT exp
    -> E f32; DVE row-sum + reciprocal; masked lanes are exactly 0.
  - attn^T via PE transposes (f32), attn @ values on PE in f32, final
    per-row 1/denom scale on the PSUM->SBUF copy.
"""

import sys

sys.path.insert(0, "/opt/trn_rl_repo")

from contextlib import ExitStack

import numpy as np

from concourse import bacc, mybir, tile
from concourse.bass_utils import run_bass_kernel_spmd
from concourse.masks import make_identity

B, Q, K, D, H, V = 4, 512, 512, 256, 128, 256
QS = Q // 2          # query rows per core
NCORES = 8
MASK_VALUE = -1000000.0

f32 = mybir.dt.float32
bf16 = mybir.dt.bfloat16

# (kind, q-count) per supertile; block 0 leads with ACT-biased q's,
# last block ramps down to shorten the serial tail.
STS0 = [("bias", 8)] + [("bat", 16)] * 7 + [("bat", 8)]
STS1 = [("bat", 16)] * 7 + [("bat", 8), ("bat", 4), ("bat", 4)]


def _build_graph(nc, tc, ctx, tensors, KC):
    p1_d, p2_d, m_d, out_d = tensors
    NKC = (KC + 127) // 128          # 128-row key chunks (last may be partial)
    WLAST = KC - (NKC - 1) * 128     # rows in last chunk
    Tanh = mybir.ActivationFunctionType.Tanh
    Exp = mybir.ActivationFunctionType.Exp
    AX = mybir.AxisListType.X
    ADD = mybir.AluOpType.add

    cpool = ctx.enter_context(tc.tile_pool(name="const", bufs=1))
    inp = ctx.enter_context(tc.tile_pool(name="inp", bufs=1))
    prep = ctx.enter_context(tc.tile_pool(name="prep", bufs=3))
    ttp = ctx.enter_context(tc.tile_pool(name="ttp", bufs=3))
    smp = ctx.enter_context(tc.tile_pool(name="smp", bufs=2))
    outp = ctx.enter_context(tc.tile_pool(name="outp", bufs=2))
    ps_big = ctx.enter_context(tc.tile_pool(name="ps_big", bufs=2, space="PSUM"))
    ps_tr = ctx.enter_context(tc.tile_pool(name="ps_tr", bufs=2, space="PSUM"))
    ps_av = ctx.enter_context(tc.tile_pool(name="ps_av", bufs=2, space="PSUM"))

    # ---------------- constants built before DVE gets busy ----------------
    ident = cpool.tile([128, 128], f32, tag="ident")
    make_identity(nc, ident[:])
    ones_f = cpool.tile([1, H], f32, tag="ones")
    nc.gpsimd.memset(ones_f[:], 1.0)
    z32 = cpool.tile([128, 1024], bf16, tag="z32")
    nc.gpsimd.memset(z32[:], 0.0)

    # ---------------- loads ----------------
    F1 = 2 * KC + 2 * 256 + 4 * 128 + 2
    pk1 = inp.tile([128, F1], f32, tag="pk1")
    nc.sync.dma_start(pk1[:], p1_d[:])
    kT_sb = [pk1[:, i * KC:(i + 1) * KC] for i in range(2)]
    o = 2 * KC
    qT_sb = [pk1[:, o + i * 256:o + (i + 1) * 256] for i in range(2)]
    o += 512
    wqT_sb = [pk1[:, o + i * 128:o + (i + 1) * 128] for i in range(2)]
    o += 256
    wkT_sb = [pk1[:, o + i * 128:o + (i + 1) * 128] for i in range(2)]
    o += 256
    wqkb = pk1[:, o:o + 1]
    wv_col = pk1[:, o + 1:o + 2]

    mask_f = inp.tile([1, KC], f32, tag="maskf")
    nc.sync.dma_start(mask_f[:], m_d[:])

    vals = inp.tile([128, NKC * V], f32, tag="vals")
    nc.sync.dma_start(vals[:], p2_d[:])

    # one-hot weighted wv columns: z32[h, j*32+m] = wv[h] iff m == j
    nc.vector.tensor_copy(z32[:, 0:1024:33], wv_col.broadcast_to([128, 32]))

    # ---------------- projections ----------------
    fk_ps = ps_big.tile([128, K], f32, tag="big", name="fk_ps")
    nc.tensor.matmul(fk_ps[:, :KC], wkT_sb[0], kT_sb[0], start=True, stop=False)
    nc.tensor.matmul(fk_ps[:, :KC], wkT_sb[1], kT_sb[1], start=False, stop=True)
    fk_sb = cpool.tile([128, KC], bf16, tag="fk_sb")
    nc.vector.tensor_copy(fk_sb[:], fk_ps[:, :KC])

    fq_ps = ps_big.tile([128, QS], f32, tag="big", name="fq_ps")
    nc.tensor.matmul(fq_ps[:], wqT_sb[0], qT_sb[0], start=True, stop=False)
    nc.tensor.matmul(fq_ps[:], wqT_sb[1], qT_sb[1], start=False, stop=True)
    fq_sb = cpool.tile([128, QS], f32, tag="fq_sb")
    # fold wq_b + wk_b into fq so the tanh input needs no extra bias
    nc.vector.tensor_scalar_add(fq_sb[:], fq_ps[:], wqkb)

    # ---------------- main loop ----------------
    for blk in range(2):
        sts = STS0 if blk == 0 else STS1
        sc_ps = ps_big.tile([128, K], f32, tag="big", name=f"sc{blk}")
        # rank-1 matmul writes mask row to every q-partition, clears bank
        nc.tensor.matmul(sc_ps[:, :KC], ones_f[:], mask_f[:], start=True,
                         stop=False, skip_group_check=True)
        r = 0
        for st, (kind, stq) in enumerate(sts):
            tt = ttp.tile([128, 16 * KC], bf16, tag="tt", name=f"tt{blk}_{st}")
            if kind == "bias":
                # ACT reads fk straight from PSUM, per-partition bias fq[:,q]
                for i in range(stq):
                    q = blk * 128 + r + i
                    nc.scalar.activation(tt[:, i * KC:(i + 1) * KC],
                                         fk_ps[:, :KC], Tanh,
                                         bias=fq_sb[:, q:q + 1])
            else:
                pre = prep.tile([128, 16 * KC], bf16, tag="pre",
                                name=f"pre{blk}_{st}")
                for i in range(stq):
                    q = blk * 128 + r + i
                    nc.vector.tensor_scalar_add(
                        pre[:, i * KC:(i + 1) * KC], fk_sb[:], fq_sb[:, q:q + 1])
                nc.scalar.activation(tt[:, :stq * KC], pre[:, :stq * KC], Tanh)
            for i in range(stq):
                g, j = divmod(r + i, 32)
                nc.tensor.matmul(
                    sc_ps[g * 32:(g + 1) * 32, :KC],
                    z32[:, j * 32:(j + 1) * 32],
                    tt[:, i * KC:(i + 1) * KC],
                    start=False, stop=(r + i == 127), skip_group_check=True,
                    tile_position=(0, g * 32))
            r += stq

        E = smp.tile([128, KC], f32, tag="E", name=f"E{blk}")
        nc.scalar.activation(E[:], sc_ps[:, :KC], Exp)
        denom = smp.tile([128, 1], f32, tag="denom", name=f"denom{blk}")
        nc.vector.tensor_reduce(denom[:], E[:], axis=AX, op=ADD)
        recip = smp.tile([128, 1], f32, tag="recip", name=f"recip{blk}")
        nc.vector.reciprocal(recip[:], denom[:])

        ET = smp.tile([128, NKC * 128], f32, tag="ET", name=f"ET{blk}")
        for ci in range(NKC):
            w = 128 if ci < NKC - 1 else WLAST
            tp = ps_tr.tile([128, 128], f32, tag="tr", name=f"tr{blk}_{ci}")
            nc.tensor.transpose(tp[:w, :], E[:, ci * 128:ci * 128 + w], ident[:])
            nc.vector.tensor_copy(ET[:w, ci * 128:(ci + 1) * 128], tp[:w, :])

        av = ps_av.tile([128, V], f32, tag="av", name=f"av{blk}")
        for ci in range(NKC):
            w = 128 if ci < NKC - 1 else WLAST
            nc.tensor.matmul(av[:], ET[:w, ci * 128:(ci + 1) * 128],
                             vals[:w, ci * V:(ci + 1) * V],
                             start=(ci == 0), stop=(ci == NKC - 1))
        osb = outp.tile([128, V], f32, tag="osb", name=f"osb{blk}")
        nc.vector.tensor_scalar_mul(osb[:], av[:], recip[:])
        nc.sync.dma_start(out_d[blk * 128:(blk + 1) * 128, :], osb[:])


def _build_kernel(KC):
    NKC = (KC + 127) // 128
    F1 = 2 * KC + 2 * 256 + 4 * 128 + 2
    nc = bacc.Bacc("TRN2", target_bir_lowering=False, debug=False,
                   num_devices=NCORES)
    p1_d = nc.dram_tensor("pack1", [128, F1], f32, kind="ExternalInput")
    p2_d = nc.dram_tensor("pack2", [128, NKC * V], f32, kind="ExternalInput")
    m_d = nc.dram_tensor("maskrow", [1, KC], f32, kind="ExternalInput")
    out_d = nc.dram_tensor("out", [QS, V], f32, kind="ExternalOutput")

    with tile.TileContext(nc) as tc, ExitStack() as ctx:
        _build_graph(nc, tc, ctx, (p1_d, p2_d, m_d, out_d), KC)
    nc.compile()
    return nc


_NC_CACHE = {}


def _get_nc(KC):
    if KC not in _NC_CACHE:
        _NC_CACHE[KC] = _build_kernel(KC)
    return _NC_CACHE[KC]


def _choose_kc(valid_lens):
    mx = int(np.max(valid_lens))
    mx = max(32, min(K, mx))
    return (mx + 31) // 32 * 32


def prepare_in_maps(queries, keys, values, valid_lens, wq_w, wq_b, wk_w,
                    wk_b, wv_w, wv_b):
    queries = np.asarray(queries, np.float32)
    keys = np.asarray(keys, np.float32)
    values = np.asarray(values, np.float32)
    wq_w = np.asarray(wq_w, np.float32)
    wq_b = np.asarray(wq_b, np.float32)
    wk_w = np.asarray(wk_w, np.float32)
    wk_b = np.asarray(wk_b, np.float32)
    wv_w = np.asarray(wv_w, np.float32)
    wv_b = np.asarray(wv_b, np.float32)
    valid_lens = np.asarray(valid_lens)

    KC = _choose_kc(valid_lens)
    NKC = (KC + 127) // 128

    wqT = wq_w.T                     # [D, H]
    wkT = wk_w.T
    wqkb = (wq_b + wk_b).reshape(H, 1)
    wv = wv_w.reshape(H, 1)

    in_maps = []
    for c in range(NCORES):
        b, half = divmod(c, 2)
        vl = int(valid_lens[b])
        mask = np.full((1, KC), MASK_VALUE, np.float32)
        mask[0, :vl] = 0.0
        mask += np.float32(wv_b.reshape(-1)[0])

        kT = keys[b, :KC, :].T                            # [D, KC]
        qT = queries[b, half * QS:(half + 1) * QS, :].T   # [D, QS]
        pack1 = np.concatenate([
            kT[0:128], kT[128:256],
            qT[0:128], qT[128:256],
            wqT[0:128], wqT[128:256],
            wkT[0:128], wkT[128:256],
            wqkb, wv,
        ], axis=1).astype(np.float32)

        vpad = np.zeros((NKC * 128, V), np.float32)
        vpad[:KC] = values[b, :KC, :]
        pack2 = np.concatenate(
            [vpad[ci * 128:(ci + 1) * 128] for ci in range(NKC)], axis=1)

        in_maps.append({
            "pack1": np.ascontiguousarray(pack1),
            "pack2": np.ascontiguousarray(pack2),
            "maskrow": mask,
        })
    return KC, in_maps


def assemble_out(results):
    out = np.empty((B, Q, V), np.float32)
    for c in range(NCORES):
        b, half = divmod(c, 2)
        out[b, half * QS:(half + 1) * QS, :] = results[c]["out"]
    return out


def kernel(**inputs):
    KC, in_maps = prepare_in_maps(**inputs)
    nc = _get_nc(KC)
    res = run_bass_kernel_spmd(nc, in_maps, list(range(NCORES))).results
    return assemble_out(res)


if __name__ == "__main__":
    rng = np.random.default_rng(0)
    inp = {
        "queries": rng.standard_normal((B, Q, D), np.float32),
        "keys": rng.standard_normal((B, K, D), np.float32),
        "values": rng.standard_normal((B, K, V), np.float32),
        "valid_lens": rng.integers(1, K + 1, (B,)).astype(np.int32),
        "wq_w": (rng.standard_normal((H, D), np.float32) / np.sqrt(D)).astype(np.float32),
        "wq_b": np.zeros((H,), np.float32),
        "wk_w": (rng.standard_normal((H, D), np.float32) / np.sqrt(D)).astype(np.float32),
        "wk_b": np.zeros((H,), np.float32),
        "wv_w": (rng.standard_normal((1, H), np.float32) / np.sqrt(H)).astype(np.float32),
        "wv_b": np.zeros((1,), np.float32),
    }
    out = kernel(**inp)
    print("kernel output", out.shape, out.dtype, float(np.abs(out).mean()))
